# revision 20
# baseline (speedup 1.0000x reference)
"""Trainium2 Bass kernel for nn_Llama3_68135361184133 (v3).

Strategy: pure data-parallel over the 112 (b,m) sequences -> 14 seqs (896
tokens) per core, 8 cores, no collectives.  Compute dtype fp16 with fp32
PSUM accumulation.  All activations stay resident in SBUF (no DRAM
bounce): slot X carries raw h1 -> h2 (in place), slot Y carries
q_T -> attention-out (in place) -> mlp delta -> h3 (in place).
Per-token RMS scales are applied post-matmul by linearity; the scale
vector is broadcast across partitions with a ones-matmul on the PE
instead of a DRAM round-trip.  MLP weights are streamed exactly once.

v3 changes vs v2 (all aimed at the ~240us of PE idle in the trace):
 - attention: 2 KV-groups per iteration, softmax denominator broadcast
   with a ones-matmul directly (no serial [1,*] psum hop), reciprocal
   via the fast custom-DVE approximation on [128,512] instead of the
   1-partition serial RECIPROCAL (1.7us each).
 - rms_finish: single Rsqrt activation replaces sqrt+reciprocal+cast.
 - P1 patch embed: bias-activation writes h_X directly (one less copy).
 - first weight slabs of each phase are DMA'd before the preceding rms
   barrier so the PE never waits on DMA at phase entry.
 - head: W_fc stored dc-major so the final-norm multiply happens
   just-in-time per dc slab and weight streaming starts during the MLP
   tail; head is HBM-bandwidth-bound so DMA never stops.
"""

import math
from contextlib import ExitStack

import numpy as np

import concourse.bass as bass
from concourse import bacc
import concourse.mybir as mybir
import concourse.tile as tile
from concourse import bass_utils
from concourse.masks import make_identity

F16 = mybir.dt.float16
F32 = mybir.dt.float32
AL = mybir.AluOpType
AF = mybir.ActivationFunctionType

B, L, M = 16, 512, 7
P_PATCH, STRIDE = 16, 8
N = 64
D, H, KV, HD, DFF = 4096, 32, 8, 128, 14336
FC, PRED = 128, 96
THETA = 500000.0
EPS = 1e-5

NCORES = 8
SEQ = 14
T = SEQ * N                # 896
C = D // 128               # 32
JC = DFF // 128            # 112
TB = 448
NTB = T // TB              # 2
JG = 14
NG = JC // JG              # 8
SPT = TB // N              # 7 sequences per token-block


class Stream:
    """Ordered weight-slab stream: prefetch issues the dma for the next
    source into a fresh pool tile; get() returns tiles in order."""

    def __init__(self, nc, pool, shape, dtype, sources, tag):
        self.nc, self.pool = nc, pool
        self.shape, self.dtype, self.tag = shape, dtype, tag
        self.sources = list(sources)
        self.idx = 0
        self.q = []

    def prefetch(self, k=1):
        for _ in range(k):
            if self.idx >= len(self.sources):
                return
            t = self.pool.tile(self.shape, self.dtype, tag=self.tag)
            self.nc.sync.dma_start(t[:], self.sources[self.idx])
            self.idx += 1
            self.q.append(t)

    def get(self):
        return self.q.pop(0)


def build_nc():
    nc = bacc.Bacc("TRN2")

    d = {}
    d["patches"] = nc.dram_tensor("patches", [128, T], F16, kind="ExternalInput")
    d["w_in"] = nc.dram_tensor("w_in", [128, C, 128], F16, kind="ExternalInput")
    d["b_in"] = nc.dram_tensor("b_in", [128, C], F32, kind="ExternalInput")
    d["wq"] = nc.dram_tensor("wq", [H, 128, C, 128], F16, kind="ExternalInput")
    d["wk"] = nc.dram_tensor("wk", [KV, 128, C, 128], F16, kind="ExternalInput")
    d["wv"] = nc.dram_tensor("wv", [KV, 128, C, 128], F16, kind="ExternalInput")
    d["wo"] = nc.dram_tensor("wo", [C, 128, C, 128], F16, kind="ExternalInput")
    d["wg"] = nc.dram_tensor("wg", [JC, 128, C, 128], F16, kind="ExternalInput")
    d["wu"] = nc.dram_tensor("wu", [JC, 128, C, 128], F16, kind="ExternalInput")
    d["wd"] = nc.dram_tensor("wd", [NG, C, 128, JG, 128], F16, kind="ExternalInput")
    d["wfc"] = nc.dram_tensor("wfc", [C, 128, N, FC], F16, kind="ExternalInput")
    d["wout"] = nc.dram_tensor("wout", [FC, PRED], F16, kind="ExternalInput")
    d["b_fc"] = nc.dram_tensor("b_fc", [1, FC], F32, kind="ExternalInput")
    d["b_out"] = nc.dram_tensor("b_out", [PRED, 1], F32, kind="ExternalInput")
    d["cos_q"] = nc.dram_tensor("cos_q", [128, N], F32, kind="ExternalInput")
    d["sin_q"] = nc.dram_tensor("sin_q", [128, N], F32, kind="ExternalInput")
    d["cos_k"] = nc.dram_tensor("cos_k", [128, N], F32, kind="ExternalInput")
    d["sin_k"] = nc.dram_tensor("sin_k", [128, N], F32, kind="ExternalInput")
    d["mask"] = nc.dram_tensor("mask", [N, N], F16, kind="ExternalInput")
    out_d = nc.dram_tensor("out", [PRED, SEQ], F32, kind="ExternalOutput")

    def tb(th):
        return slice(th * TB, (th + 1) * TB)

    with tile.TileContext(nc) as tc, ExitStack() as ctx:
        consts = ctx.enter_context(tc.tile_pool(name="consts", bufs=1))
        scl = ctx.enter_context(tc.tile_pool(name="scl", bufs=1))
        big = ctx.enter_context(tc.tile_pool(name="big", bufs=1))
        # one shared stream pool for all [128, C, 128] weight slabs so a
        # phase's first slabs can be DMA'd during the previous phase
        wstream = ctx.enter_context(tc.tile_pool(name="wstream", bufs=3))
        es1 = ExitStack()
        wsl1 = es1.enter_context(tc.tile_pool(name="wsl1", bufs=1))

        # inputs on the P1 critical path first
        patches_sb = wsl1.tile([128, T], F16)
        nc.sync.dma_start(patches_sb[:], d["patches"][:])
        w_in_sb = wsl1.tile([128, C, 128], F16, tag="win")
        nc.sync.dma_start(w_in_sb[:], d["w_in"][:])
        b_in_sb = consts.tile([128, C], F32)
        nc.sync.dma_start(b_in_sb[:], d["b_in"][:])

        cosq = consts.tile([128, N], F32)
        nc.sync.dma_start(cosq[:], d["cos_q"][:])
        sinq = consts.tile([128, N], F32)
        nc.sync.dma_start(sinq[:], d["sin_q"][:])
        cosk = consts.tile([128, N], F32)
        nc.sync.dma_start(cosk[:], d["cos_k"][:])
        sink = consts.tile([128, N], F32)
        nc.sync.dma_start(sink[:], d["sin_k"][:])
        mask_sb = consts.tile([N, N], F16)
        nc.sync.dma_start(mask_sb[:], d["mask"][:])
        ones128 = consts.tile([128, 1], F16)
        nc.vector.memset(ones128[:], 1.0)
        ones1 = consts.tile([1, 128], F16)
        nc.vector.memset(ones1[:], 1.0)
        ones2d = consts.tile([128, 128], F16)
        nc.vector.memset(ones2d[:], 1.0)
        ident = consts.tile([128, 128], F16)
        make_identity(nc, ident[:])
        wout_sb = consts.tile([FC, PRED], F16)
        nc.sync.dma_start(wout_sb[:], d["wout"][:])
        bfc_sb = consts.tile([SEQ, FC], F32)
        nc.sync.dma_start(bfc_sb[:], d["b_fc"][:].to_broadcast((SEQ, FC)))
        bout_sb = consts.tile([PRED, 1], F32)
        nc.sync.dma_start(bout_sb[:], d["b_out"][:])
        eps_sb = consts.tile([1, 1], F32)
        nc.vector.memset(eps_sb[:], EPS)

        ss_sb = scl.tile([1, T], F32, tag="ss")
        sv32 = scl.tile([1, T], F32, tag="sv32")
        svr = scl.tile([1, T], F32, tag="svr")
        sv16 = scl.tile([1, T], F16, tag="sv16")
        s_rep = scl.tile([128, T], F16, tag="srep")

        def rms_sv():
            # sv = 1/sqrt(mean(h^2) + eps)
            nc.scalar.activation(sv32[:], ss_sb[:], AF.Sqrt, bias=eps_sb[:],
                                 scale=1.0 / D)
            nc.vector.reciprocal_approx_fast(svr[:], sv32[:])
            nc.vector.tensor_copy(sv16[:], svr[:])

        def rms_bcast(ph, bufs=2):
            # broadcast sv to all partitions via PE ones-matmul
            with tc.tile_pool(name=f"bc{ph}", bufs=bufs, space="PSUM") as bp:
                for th in range(NTB):
                    pb = bp.tile([128, TB], F32, tag="b")
                    nc.tensor.matmul(pb[:], ones1[:], sv16[:, tb(th)],
                                     start=True, stop=True)
                    nc.vector.tensor_copy(s_rep[:, tb(th)], pb[:])

        def rms_finish(ph):
            rms_sv()
            rms_bcast(ph)

        # slot X: h1 raw f16 -> h2 raw f16 (in place)
        # slot Y: q_T -> attention out (in place) -> delta -> h3 (in place)
        h_X = big.tile([128, C, T], F16, tag="X", name="h_X")

        # ---------------- P1: patch embed ----------------
        qkv_src = ([d["wq"][f] for f in range(H)] + [d["wk"][f] for f in range(KV)]
                   + [d["wv"][f] for f in range(KV)])
        st2 = Stream(nc, wstream, [128, C, 128], F16, qkv_src, "w")

        with ExitStack() as s1:
            st = s1.enter_context(tc.tile_pool(name="st1", bufs=3))
            ps = s1.enter_context(tc.tile_pool(name="ps1", bufs=2, space="PSUM"))
            nc.vector.memset(ss_sb[:], 0.0)
            for c in range(C):
                pse = [ps.tile([128, TB], F32, tag=f"mm{th}", name=f"pse{th}")
                       for th in range(NTB)]
                for th in range(NTB):
                    nc.tensor.matmul(pse[th][:], w_in_sb[:, c, :],
                                     patches_sb[:, tb(th)], start=True, stop=True)
                for th in range(NTB):
                    nc.scalar.activation(h_X[:, c, tb(th)], pse[th][:], AF.Identity,
                                         bias=b_in_sb[:, c:c + 1])
                    sqt = st.tile([128, TB], F16, tag="sqt")
                    nc.vector.tensor_tensor(sqt[:], h_X[:, c, tb(th)],
                                            h_X[:, c, tb(th)], AL.mult)
                    pss = ps.tile([1, TB], F32, tag="ssp")
                    nc.tensor.matmul(pss[:], ones128[:], sqt[:], start=True, stop=True)
                    nc.vector.tensor_add(ss_sb[:, tb(th)], ss_sb[:, tb(th)], pss[:])
        es1.close()                # frees patches + w_in space
        st2.prefetch(2)            # wq[0], wq[1] land during the rms barrier
        rms_sv()

        # ---------------- P2 + P3 ----------------
        q_T = big.tile([128, H, T], F16, tag="Y", name="q_T")
        wo_src = [d["wo"][f] for f in range(C)]
        with ExitStack() as s23:
            kv = s23.enter_context(tc.tile_pool(name="kv", bufs=1))
            k_T = kv.tile([128, KV, T], F16, tag="k")
            v_N = kv.tile([64, SEQ, KV, HD], F16, tag="v")

            with ExitStack() as s2:
                rt = s2.enter_context(tc.tile_pool(name="rt2", bufs=2))
                ps = s2.enter_context(tc.tile_pool(name="ps2", bufs=2, space="PSUM"))
                pst = s2.enter_context(tc.tile_pool(name="pst2", bufs=2, space="PSUM"))

                def emit_proj_mm(slab):
                    pp = [ps.tile([128, TB], F32, tag=f"mm{th}", name=f"pp{th}")
                          for th in range(NTB)]
                    for c in range(C):
                        for th in range(NTB):
                            nc.tensor.matmul(pp[th][:], slab[:, c, :],
                                             h_X[:, c, tb(th)],
                                             start=(c == 0), stop=(c == C - 1))
                    return pp

                def rope_post(pp, f, dst, cos_t, sin_t):
                    for th in range(NTB):
                        p = pp[th]
                        pv = p[:].rearrange("p (s n) -> p s n", n=N)
                        t1 = rt.tile([128, TB], F32, tag="t1")
                        t2 = rt.tile([128, TB], F32, tag="t2")
                        t1v = t1[:].rearrange("p (s n) -> p s n", n=N)
                        t2v = t2[:].rearrange("p (s n) -> p s n", n=N)
                        cb = cos_t[:, None, :].to_broadcast((128, SPT, N))
                        nc.vector.tensor_tensor(t1v, pv, cb, AL.mult)
                        slo = sin_t[0:64][:, None, :].to_broadcast((64, SPT, N))
                        shi = sin_t[64:128][:, None, :].to_broadcast((64, SPT, N))
                        nc.vector.tensor_tensor(t2v[0:64], pv[64:128], slo, AL.mult)
                        nc.vector.tensor_tensor(t2v[64:128], pv[0:64], shi, AL.mult)
                        nc.vector.tensor_add(t1[:], t1[:], t2[:])
                        nc.vector.tensor_tensor(dst[:, f, tb(th)], t1[:],
                                                s_rep[:, tb(th)], AL.mult)

                def proj_rope(nf, dst, cos_t, sin_t, pp_first=None):
                    for f in range(nf):
                        if f == 0 and pp_first is not None:
                            pp = pp_first
                        else:
                            slab = st2.get()
                            pp = emit_proj_mm(slab)
                            st2.prefetch(1)
                        rope_post(pp, f, dst, cos_t, sin_t)

                # overlap the rms1 broadcast with the first q slab's matmuls
                slab0 = st2.get()
                pp0 = emit_proj_mm(slab0)
                rms_bcast(1, bufs=1)
                st2.prefetch(1)
                proj_rope(H, q_T, cosq, sinq, pp_first=pp0)
                proj_rope(KV, k_T, cosk, sink)

                for f in range(KV):
                    slab = st2.get()
                    pp = [ps.tile([128, TB], F32, tag=f"mm{th}", name=f"ppv{th}")
                          for th in range(NTB)]
                    for c in range(C):
                        for th in range(NTB):
                            nc.tensor.matmul(pp[th][:], slab[:, c, :],
                                             h_X[:, c, tb(th)],
                                             start=(c == 0), stop=(c == C - 1))
                    st2.prefetch(1)
                    for th in range(NTB):
                        p = pp[th]
                        vt = rt.tile([128, TB], F16, tag="vt")
                        nc.vector.tensor_tensor(vt[:], p[:], s_rep[:, tb(th)], AL.mult)
                        for si in range(SPT):
                            s = th * SPT + si
                            ptr = pst.tile([N, 128], F16, tag="tr")
                            nc.tensor.transpose(ptr[:], vt[:, si * N:(si + 1) * N],
                                                ident[:])
                            nc.vector.tensor_copy(v_N[:, s, f, :], ptr[:])

            # P3: attention; output written in place over q_T
            o_T = q_T
            st4 = Stream(nc, wstream, [128, C, 128], F16, wo_src, "w")
            st4.prefetch(2)        # wo[0..1] stream in during attention
            NGG = KV // 2          # 2 KV groups per iteration
            with ExitStack() as s3:
                at = s3.enter_context(tc.tile_pool(name="at3", bufs=3))
                ps = s3.enter_context(tc.tile_pool(name="ps3", bufs=2, space="PSUM"))

                def attn_tail(s, g2, pt_sb):
                    # denom broadcast + P@V + normalize for a finished pair
                    sl = slice(s * N, (s + 1) * N)
                    ptf = pt_sb[:].rearrange("k a h n -> k (a h n)")
                    psb = ps.tile([128, 2, 4, N], F32, tag="pb")
                    nc.tensor.matmul(psb[:].rearrange("p a h n -> p (a h n)"),
                                     ones2d[0:N, :], ptf, start=True, stop=True)
                    rb = at.tile([128, 2, 4, N], F32, tag="rb")
                    nc.vector.reciprocal_approx_fast(
                        rb[:].rearrange("p a h n -> p (a h n)"),
                        psb[:].rearrange("p a h n -> p (a h n)"))
                    pso = ps.tile([128, 2, 4, N], F32, tag="po")
                    for gg in range(2):
                        g = 2 * g2 + gg
                        nc.tensor.matmul(
                            pso[:, gg].rearrange("p h n -> p (h n)"),
                            v_N[:, s, g, :],
                            ptf[:, gg * 4 * N:(gg + 1) * 4 * N],
                            start=True, stop=True)
                    ov = o_T[:, 8 * g2:8 * g2 + 8, sl]
                    pso3 = pso[:].rearrange("p a h n -> p (a h) n")
                    rb3 = rb[:].rearrange("p a h n -> p (a h) n")
                    nc.vector.tensor_tensor(ov, pso3, rb3, AL.mult)

                # software-pipelined: emit scores(i) before tail(i-1) so the
                # PE always has ready work while scalar/vector process i-1
                prev = None
                for s in range(SEQ):
                    sl = slice(s * N, (s + 1) * N)
                    for g2 in range(NGG):
                        psp = ps.tile([N, 2, 4, N], F32, tag="pp")
                        for gg in range(2):
                            g = 2 * g2 + gg
                            nc.tensor.matmul(psp[:, gg], k_T[:, g, sl],
                                             q_T[:, 4 * g:4 * g + 4, sl],
                                             start=True, stop=True)
                        if prev is not None:
                            attn_tail(*prev)
                        pt_sb = at.tile([N, 2, 4, N], F16, tag="pt")
                        nc.scalar.activation(pt_sb[:], psp[:], AF.Exp)
                        mb = mask_sb[:, None, None, :].to_broadcast((N, 2, 4, N))
                        nc.vector.tensor_tensor(pt_sb[:], pt_sb[:], mb, AL.mult)
                        prev = (s, g2, pt_sb)
                attn_tail(*prev)

        # ---------------- P4: Wo + residual (h2 overwrites h1 in slot X) ----
        gu_src = []
        for j in range(JC):
            gu_src.append(d["wg"][j])
            gu_src.append(d["wu"][j])
        st5 = Stream(nc, wstream, [128, C, 128], F16, gu_src, "w")

        with ExitStack() as s4:
            st = s4.enter_context(tc.tile_pool(name="st4", bufs=3))
            ps = s4.enter_context(tc.tile_pool(name="ps4", bufs=2, space="PSUM"))
            nc.vector.memset(ss_sb[:], 0.0)
            for f in range(C):
                slab = st4.get()
                pp = [ps.tile([128, TB], F32, tag=f"mm{th}", name=f"ppo{th}")
                      for th in range(NTB)]
                for c in range(C):
                    for th in range(NTB):
                        nc.tensor.matmul(pp[th][:], slab[:, c, :], o_T[:, c, tb(th)],
                                         start=(c == 0), stop=(c == C - 1))
                st4.prefetch(1)
                for th in range(NTB):
                    p = pp[th]
                    h2t = st.tile([128, TB], F32, tag="h2t")
                    nc.vector.tensor_add(h2t[:], p[:], h_X[:, f, tb(th)])
                    nc.vector.tensor_copy(h_X[:, f, tb(th)], h2t[:])
                    sqt = st.tile([128, TB], F16, tag="sqt")
                    nc.vector.tensor_tensor(sqt[:], h_X[:, f, tb(th)],
                                            h_X[:, f, tb(th)], AL.mult)
                    pss = ps.tile([1, TB], F32, tag="ssp")
                    nc.tensor.matmul(pss[:], ones128[:], sqt[:], start=True, stop=True)
                    nc.vector.tensor_add(ss_sb[:, tb(th)], ss_sb[:, tb(th)], pss[:])
        st5.prefetch(2)            # wg[0], wu[0] land during the rms barrier
        rms_sv()

        # ---------------- P5: SwiGLU MLP (weights streamed once) ----------
        delta16 = big.tile([128, C, NTB, TB], F16, tag="Y", name="delta16")
        with ExitStack() as s5:
            agp = s5.enter_context(tc.tile_pool(name="ag5", bufs=1))
            wdl = s5.enter_context(tc.tile_pool(name="wdl5", bufs=2))
            mt = s5.enter_context(tc.tile_pool(name="mt5", bufs=3))
            st = s5.enter_context(tc.tile_pool(name="st5", bufs=2))
            ps = s5.enter_context(tc.tile_pool(name="ps5", bufs=1, space="PSUM"))
            stwd = Stream(nc, wdl, [128, JG, 128], F16,
                          [d["wd"][g, f] for g in range(NG) for f in range(C)], "wd")
            first_j = True
            for g in range(NG):
                a_g = agp.tile([128, JG, NTB, TB], F16, tag="ag")
                for jj in range(JG):
                    gs = st5.get()
                    us = st5.get()
                    psg = [ps.tile([128, TB], F32, tag=f"g{th}", name=f"psg{th}")
                           for th in range(NTB)]
                    for c in range(C):
                        for th in range(NTB):
                            nc.tensor.matmul(psg[th][:], gs[:, c, :],
                                             h_X[:, c, tb(th)],
                                             start=(c == 0), stop=(c == C - 1))
                    psu = [ps.tile([128, TB], F32, tag=f"u{th}", name=f"psu{th}")
                           for th in range(NTB)]
                    for c in range(C):
                        for th in range(NTB):
                            nc.tensor.matmul(psu[th][:], us[:, c, :],
                                             h_X[:, c, tb(th)],
                                             start=(c == 0), stop=(c == C - 1))
                    if first_j:
                        # rms2 broadcast hides behind the first gu matmuls
                        rms_bcast(2, bufs=1)
                        nc.vector.memset(ss_sb[:], 0.0)
                        first_j = False
                    st5.prefetch(2)
                    for th in range(NTB):
                        t1 = mt.tile([128, TB], F16, tag="gt")
                        nc.vector.tensor_tensor(t1[:], psg[th][:], s_rep[:, tb(th)],
                                                AL.mult)
                        sg = mt.tile([128, TB], F16, tag="sg")
                        nc.scalar.activation(sg[:], t1[:], AF.Silu)
                        nc.vector.tensor_tensor(a_g[:, jj, th, :], sg[:], psu[th][:],
                                                AL.mult)
                if g == 0:
                    stwd.prefetch(2)
                for f in range(C):
                    dsl = stwd.get()
                    psd = [ps.tile([128, TB], F32, tag=f"d{th}", name=f"psd{th}")
                           for th in range(NTB)]
                    for jj in range(JG):
                        for th in range(NTB):
                            nc.tensor.matmul(psd[th][:], dsl[:, jj, :],
                                             a_g[:, jj, th, :],
                                             start=(jj == 0), stop=(jj == JG - 1))
                    stwd.prefetch(1)
                    for th in range(NTB):
                        if g == 0:
                            nc.vector.tensor_copy(delta16[:, f, th, :], psd[th][:])
                        else:
                            nc.vector.tensor_add(delta16[:, f, th, :],
                                                 delta16[:, f, th, :], psd[th][:])
            # h3 = h2 + delta * s2 ; stats for final norm; h3 overwrites delta
            with tc.tile_pool(name="ps5b", bufs=2, space="PSUM") as ps2:
                for f in range(C):
                    for th in range(NTB):
                        dt = st.tile([128, TB], F32, tag="dt")
                        nc.vector.tensor_tensor(dt[:], delta16[:, f, th, :],
                                                s_rep[:, tb(th)], AL.mult)
                        nc.vector.tensor_add(delta16[:, f, th, :], dt[:],
                                             h_X[:, f, tb(th)])
                        sqt = st.tile([128, TB], F16, tag="sqt")
                        nc.vector.tensor_tensor(sqt[:], delta16[:, f, th, :],
                                                delta16[:, f, th, :], AL.mult)
                        pss = ps2.tile([1, TB], F32, tag="ssp")
                        nc.tensor.matmul(pss[:], ones128[:], sqt[:],
                                         start=True, stop=True)
                        nc.vector.tensor_add(ss_sb[:, tb(th)], ss_sb[:, tb(th)],
                                             pss[:])

        # ---------------- P6: head (dc-major, JIT final norm) -------------
        with ExitStack() as s6:
            wsl6 = s6.enter_context(tc.tile_pool(name="wsl6", bufs=3))
            st = s6.enter_context(tc.tile_pool(name="st6", bufs=2))
            ps = s6.enter_context(tc.tile_pool(name="ps6", bufs=1, space="PSUM"))
            st6 = Stream(nc, wsl6, [128, N, FC], F16,
                         [d["wfc"][f] for f in range(C)], "wfc")
            st6.prefetch(2)        # stream W_fc during the h3 tail + rms
            rms_finish(3)
            h3n_r = (delta16[:].rearrange("p c t b -> p c (t b)")
                     .rearrange("p c (s n) -> p c n s", n=N))
            psz = ps.tile([SEQ, FC], F32, tag="z")
            for f in range(C):
                # final norm just-in-time for this dc slab
                for th in range(NTB):
                    nc.vector.tensor_tensor(delta16[:, f, th, :],
                                            delta16[:, f, th, :],
                                            s_rep[:, tb(th)], AL.mult)
                slab = st6.get()
                st6.prefetch(1)
                for t in range(N):
                    nc.tensor.matmul(psz[:], h3n_r[:, f, t, :], slab[:, t, :],
                                     start=(f == 0 and t == 0),
                                     stop=(f == C - 1 and t == N - 1))
            z1 = st.tile([SEQ, FC], F32, tag="z1")
            nc.vector.tensor_add(z1[:], psz[:], bfc_sb[:])
            zl = st.tile([SEQ, FC], F16, tag="zl")
            nc.scalar.activation(zl[:], z1[:], AF.Lrelu, alpha=0.01)
            pzt = ps.tile([FC, SEQ], F16, tag="zt")
            nc.tensor.transpose(pzt[:], zl[:], ident[0:SEQ, 0:SEQ])
            zT = st.tile([FC, SEQ], F16, tag="zT")
            nc.vector.tensor_copy(zT[:], pzt[:])
            ps2o = ps.tile([PRED, SEQ], F32, tag="o2")
            nc.tensor.matmul(ps2o[:], wout_sb[:], zT[:], start=True, stop=True)
            osb = st.tile([PRED, SEQ], F32, tag="osb")
            nc.vector.tensor_tensor(
                osb[:], ps2o[:],
                bout_sb[:, 0:1].to_broadcast((PRED, SEQ)), AL.add)
            nc.sync.dma_start(out_d[:], osb[:])

    nc.finalize()
    _dedup_ldweights(nc)
    return nc


def _dedup_ldweights(nc):
    """Drop InstLdweights whose weights AP equals the previous load on the
    PE stream (the PE array already holds those weights).  Only removes
    loads that carry no semaphore waits/updates, so scheduling is
    unaffected."""
    total = 0
    for b in nc.m.functions[0].blocks:
        il = b.instructions
        keep = []
        last_w = None
        removed = 0
        for inst in il:
            tn = type(inst).__name__
            if tn == "InstLdweights":
                apstr = str(inst.ins[0])
                if apstr == last_w and not inst.has_wait() and not inst.has_update():
                    removed += 1
                    continue
                last_w = apstr
            elif tn == "InstMatmult":
                if inst.ldweights:
                    last_w = None
            keep.append(inst)
        if removed:
            il[:] = keep
            total += removed
    return total


# ---------------- host side ----------------

def _prep_weights(inputs):
    f16 = np.float16
    anw = np.asarray(inputs["attn_norm_w"], np.float32)
    mnw = np.asarray(inputs["mlp_norm_w"], np.float32)
    fnw = np.asarray(inputs["final_norm_w"], np.float32)

    def swz(wT, nf):  # [Din, nf*128] -> [nf, 128ci, C, 128m]
        return np.ascontiguousarray(
            wT.reshape(C, 128, nf, 128).transpose(2, 1, 0, 3).astype(f16))

    w = {}
    w["w_in"] = np.zeros((128, C, 128), f16)
    w["w_in"][:P_PATCH] = (np.asarray(inputs["W_in"], np.float32).T
                           .reshape(P_PATCH, C, 128).astype(f16))
    w["b_in"] = np.ascontiguousarray(
        np.asarray(inputs["b_in"], np.float32).reshape(C, 128).T)
    w["wq"] = swz((np.asarray(inputs["Wq"], np.float32) * anw[None, :]).T, H)
    w["wk"] = swz((np.asarray(inputs["Wk"], np.float32) * anw[None, :]).T, KV)
    w["wv"] = swz((np.asarray(inputs["Wv"], np.float32) * anw[None, :]).T, KV)
    w["wo"] = swz(np.asarray(inputs["Wo"], np.float32).T, C)
    w["wg"] = swz((np.asarray(inputs["Wg"], np.float32) * mnw[None, :]).T, JC)
    w["wu"] = swz((np.asarray(inputs["Wu"], np.float32) * mnw[None, :]).T, JC)
    wdT = np.asarray(inputs["Wd"], np.float32).T          # [DFF, D]
    wd5 = wdT.reshape(NG, JG, 128, C, 128)                # [g, jj, ji, f, m]
    w["wd"] = np.ascontiguousarray(wd5.transpose(0, 3, 2, 1, 4).astype(f16))
    wfcT = (np.asarray(inputs["W_fc"], np.float32).reshape(FC, N, D)
            * fnw[None, None, :]).reshape(FC, N * D).T    # [N*D, FC]
    w["wfc"] = np.ascontiguousarray(
        wfcT.reshape(N, C, 128, FC).transpose(1, 2, 0, 3).astype(f16))
    w["wout"] = np.ascontiguousarray(
        np.asarray(inputs["W_out"], np.float32).T.astype(f16))
    w["b_fc"] = np.asarray(inputs["b_fc"], np.float32).reshape(1, FC).copy()
    w["b_out"] = np.asarray(inputs["b_out"], np.float32).reshape(PRED, 1).copy()

    inv_freq = 1.0 / (THETA ** (np.arange(0, HD, 2, dtype=np.float32) / HD))
    ang = np.arange(N, dtype=np.float32)[:, None] * inv_freq[None, :]
    cos_h = np.cos(ang).T.astype(np.float32)              # [64, N]
    sin_h = np.sin(ang).T.astype(np.float32)
    cos_t = np.concatenate([cos_h, cos_h], 0)
    sin_t = np.concatenate([-sin_h, sin_h], 0)            # sign-folded
    sc = 1.0 / math.sqrt(HD)
    w["cos_q"] = np.ascontiguousarray(cos_t * sc)
    w["sin_q"] = np.ascontiguousarray(sin_t * sc)
    w["cos_k"] = np.ascontiguousarray(cos_t)
    w["sin_k"] = np.ascontiguousarray(sin_t)
    kk = np.arange(N)[:, None]
    qq = np.arange(N)[None, :]
    w["mask"] = np.ascontiguousarray((kk <= qq).astype(f16))
    return w


_NC_CACHE = {}


def kernel(**inputs) -> np.ndarray:
    x = np.asarray(inputs["x"], np.float32)
    means = x.mean(axis=1, keepdims=True)                 # (16, 1, 7)
    stdev = np.sqrt(x.var(axis=1) + EPS)                  # (16, 7)
    xn = (x - means) / stdev[:, None, :]
    xt = xn.transpose(0, 2, 1).reshape(B * M, L)
    xp = np.concatenate([xt, np.repeat(xt[:, -1:], STRIDE, 1)], 1)
    idx = np.arange(N)[:, None] * STRIDE + np.arange(P_PATCH)[None, :]
    patches = xp[:, idx]                                  # (112, 64, 16)

    w = _prep_weights(inputs)

    if "nc" not in _NC_CACHE:
        _NC_CACHE["nc"] = build_nc()
    nc = _NC_CACHE["nc"]

    in_maps = []
    for core in range(NCORES):
        pc = patches[core * SEQ:(core + 1) * SEQ]
        pt = np.zeros((128, T), np.float16)
        pt[:P_PATCH] = pc.reshape(T, P_PATCH).T.astype(np.float16)
        m = dict(w)
        m["patches"] = pt
        in_maps.append(m)

    res = bass_utils.run_bass_kernel_spmd(nc, in_maps, core_ids=list(range(NCORES)))

    out = np.zeros((B, PRED, M), np.float32)
    for core in range(NCORES):
        oc = res.results[core]["out"]                     # (96, 14)
        for sl in range(SEQ):
            s = core * SEQ + sl
            b, mi = divmod(s, M)
            out[b, :, mi] = oc[:, sl] * stdev[b, mi] + means[b, 0, mi]
    return out
